# revision 5
# baseline (speedup 1.0000x reference)
"""Trainium2 Bass kernel for nn_CategoricalLinear (MoE-routing batched matvec).

Problem: out[b] = weight[selected_ids[b]] @ x[b]
  x: [2048, 512] f32, selected_ids: [2048] int, weight: [64, 512, 512] f32
  out: [2048, 512] f32

Strategy (category-sharded, NOT the data-parallel hint):
  - Host: stable-sort samples by category; category c's samples become a
    contiguous block. Transpose x so features lie on SBUF partitions.
  - Each of the 8 cores owns 8 categories (8 MB weight slab — the minimal
    1/8 slice of the 64 MB table) and ALL samples routed to them (~256).
  - Per category g: out_g[s, o] = sum_i x[s, i] * W_g[o, i] computed as
    4 accumulating PE matmuls: stationary = xT chunk [128(K=IN), PC(samples)],
    moving = W_g^T chunk [128(K=IN), 512(OUT)], PSUM [PC, 512].
    float32r data path -> full-rate PE (fp32 would stream at 1/4 rate).
  - Weight slab streamed per-category (1 MB DMAs) and double-buffered so the
    PE and the output path hide entirely under the weight DMA (~8 MB/core,
    the bandwidth floor for this sharding).
  - Host: unpad + inverse-permute rows back to the original sample order.

This is better than data-parallel replication: sharding the batch would make
every core read ~the whole 64 MB table (8x the aggregate HBM traffic) and
leaves ~4 samples per (core, category) matmul.
"""

import numpy as np

B, IN, OUT, C = 2048, 512, 512, 64
NCORES = 8
CPC = C // NCORES  # categories per core
KCH = IN // 128  # contraction chunks of 128


def _build_nc(
    PC: int,
    mm_dtype: str = "float32r",
    loop_iters: int = 0,
    unroll: int = 1,
    wbufs: int = 4,
):
    """Build + compile the SPMD Bass program (same NEFF runs on all 8 cores).

    PC: per-category sample capacity (padded), <= 128.
    loop_iters: if > 0, wrap the body in a device-side For_i loop with
        `unroll` copies of the body per iteration (timing use only).
    """
    import concourse.mybir as mybir
    import concourse.tile as tile
    from concourse import bacc

    f32 = mybir.dt.float32
    mmdt = getattr(mybir.dt, mm_dtype)

    nc = bacc.Bacc(
        "TRN2", target_bir_lowering=False, debug=False, num_devices=NCORES
    )
    wt = nc.dram_tensor("wt", [CPC * IN, OUT], mmdt, kind="ExternalInput").ap()
    xt = nc.dram_tensor("xt", [IN, CPC * PC], mmdt, kind="ExternalInput").ap()
    out = nc.dram_tensor("out", [CPC * PC, OUT], f32, kind="ExternalOutput").ap()

    with tile.TileContext(nc) as tc:
        with (
            tc.tile_pool(name="xp", bufs=1) as xp,
            tc.tile_pool(name="wp", bufs=wbufs) as wp,
            tc.tile_pool(name="pp", bufs=4, space="PSUM") as pp,
            tc.tile_pool(name="op", bufs=3) as op,
        ):

            def body():
                # x^T resident in SBUF: 4 chunks of [128, CPC*PC]
                xts = []
                for k in range(KCH):
                    t = xp.tile([128, CPC * PC], mmdt, tag=f"x{k}")
                    nc.sync.dma_start(out=t[:], in_=xt[k * 128 : (k + 1) * 128, :])
                    xts.append(t)
                for g in range(CPC):
                    # W_g^T as [IN, OUT] -> SBUF [128, KCH, OUT]; partition p is
                    # IN row k*128+p, free dims (k, o). 1 MB DMA, 2 KB runs.
                    wtile = wp.tile([128, KCH, OUT], mmdt)
                    src = wt[g * IN : (g + 1) * IN, :].rearrange(
                        "(k p) o -> p k o", p=128
                    )
                    nc.sync.dma_start(out=wtile[:], in_=src)
                    ps = pp.tile([PC, OUT], f32)
                    for k in range(KCH):
                        nc.tensor.matmul(
                            ps[:],
                            lhsT=xts[k][:, g * PC : (g + 1) * PC],
                            rhs=wtile[:, k, :],
                            start=(k == 0),
                            stop=(k == KCH - 1),
                        )
                    ot = op.tile([PC, OUT], f32)
                    nc.vector.tensor_copy(out=ot[:], in_=ps[:])
                    nc.scalar.dma_start(out=out[g * PC : (g + 1) * PC, :], in_=ot[:])

            if loop_iters > 0:
                with tc.For_i(0, loop_iters, 1):
                    for _ in range(unroll):
                        body()
            else:
                body()
    nc.compile()
    return nc


def _prepare(x, selected_ids, weight):
    """Host-side shard prep. Returns (in_maps, meta) or (None, fallback_out)."""
    x = np.ascontiguousarray(np.asarray(x, dtype=np.float32))
    ids = np.asarray(selected_ids).astype(np.int64).ravel()
    weight = np.asarray(weight, dtype=np.float32)
    counts = np.bincount(ids, minlength=C)
    mx = int(counts.max())
    if mx > 128 or weight.shape != (C, OUT, IN) or x.shape != (B, IN):
        return None, None  # pathological skew / unexpected shape -> host path
    order = np.argsort(ids, kind="stable")
    x_sorted = x[order]
    offs = np.zeros(C + 1, np.int64)
    offs[1:] = np.cumsum(counts)
    PC = max(16, -(-mx // 16) * 16)  # round up to 16, at least 16
    wt_t = np.ascontiguousarray(weight.transpose(0, 2, 1))  # [C, IN, OUT]
    in_maps = []
    for core in range(NCORES):
        xt_k = np.zeros((IN, CPC * PC), np.float32)
        for gl in range(CPC):
            c = core * CPC + gl
            n = int(counts[c])
            if n:
                xt_k[:, gl * PC : gl * PC + n] = x_sorted[offs[c] : offs[c + 1]].T
        w_k = wt_t[core * CPC : (core + 1) * CPC].reshape(CPC * IN, OUT)
        in_maps.append({"wt": w_k, "xt": xt_k})
    meta = dict(PC=PC, counts=counts, offs=offs, order=order)
    return in_maps, meta


def _gather(results, meta):
    counts, offs, order, PC = (
        meta["counts"],
        meta["offs"],
        meta["order"],
        meta["PC"],
    )
    out_sorted = np.empty((B, OUT), np.float32)
    for core in range(NCORES):
        o = results[core]["out"]
        for gl in range(CPC):
            c = core * CPC + gl
            n = int(counts[c])
            if n:
                out_sorted[offs[c] : offs[c + 1]] = o[gl * PC : gl * PC + n]
    out_full = np.empty_like(out_sorted)
    out_full[order] = out_sorted
    return out_full


def kernel(x, selected_ids, weight):
    in_maps, meta = _prepare(x, selected_ids, weight)
    if in_maps is None:
        ids = np.asarray(selected_ids).astype(np.int64).ravel()
        w = np.asarray(weight, dtype=np.float32)
        xx = np.asarray(x, dtype=np.float32)
        return np.einsum("boi,bi->bo", w[ids], xx).astype(np.float32)
    from concourse.bass_utils import run_bass_kernel_spmd

    nc = _build_nc(meta["PC"])
    res = run_bass_kernel_spmd(nc, in_maps, core_ids=list(range(NCORES)))
    return _gather(res.results, meta)


# revision 11
# speedup vs baseline: 1.2879x; 1.2879x over previous
"""Trainium2 Bass kernel for nn_CategoricalLinear (MoE-routing batched matvec).

Problem: out[b] = weight[selected_ids[b]] @ x[b]
  x: [2048, 512] f32, selected_ids: [2048] int, weight: [64, 512, 512] f32
  out: [2048, 512] f32

Strategy (category-sharded, NOT the data-parallel hint):
  - Host: stable-sort samples by category; category c's samples become a
    contiguous block. Transpose x so features lie on SBUF partitions.
  - Each of the 8 cores owns 8 categories (8 MB weight slab — the minimal
    1/8 slice of the 64 MB table) and ALL samples routed to them (~256).
  - Per category g: out_g[s, o] = sum_i x[s, i] * W_g[o, i] computed as
    4 accumulating PE matmuls: stationary = xT chunk [128(K=IN), PC(samples)],
    moving = W_g^T chunk [128(K=IN), 512(OUT)], PSUM [PC, 512].
    float32r data path -> full-rate PE (fp32 would stream at 1/4 rate).
  - Weight slab streamed per-category (1 MB DMAs) and double-buffered so the
    PE and the output path hide entirely under the weight DMA (~8 MB/core,
    the bandwidth floor for this sharding).
  - Host: unpad + inverse-permute rows back to the original sample order.

This is better than data-parallel replication: sharding the batch would make
every core read ~the whole 64 MB table (8x the aggregate HBM traffic) and
leaves ~4 samples per (core, category) matmul.
"""

import numpy as np

B, IN, OUT, C = 2048, 512, 512, 64
NCORES = 8
CPC = C // NCORES  # categories per core
KCH = IN // 128  # contraction chunks of 128


def _build_nc(
    PC: int,
    mm_dtype: str = "float32r",
    loop_iters: int = 0,
    unroll: int = 1,
    wbufs: int = 4,
    cats_per_dma: int = 1,
    interleave: bool = False,
):
    """Build + compile the SPMD Bass program (same NEFF runs on all 8 cores).

    PC: per-category sample capacity (padded), <= 128.
    loop_iters: if > 0, wrap the body in a device-side For_i loop with
        `unroll` copies of the body per iteration (timing use only).
    """
    import concourse.mybir as mybir
    import concourse.tile as tile
    from concourse import bacc

    f32 = mybir.dt.float32
    mmdt = getattr(mybir.dt, mm_dtype)

    nc = bacc.Bacc(
        "TRN2", target_bir_lowering=False, debug=False, num_devices=NCORES
    )
    wt = nc.dram_tensor("wt", [CPC * IN, OUT], mmdt, kind="ExternalInput").ap()
    xt = nc.dram_tensor("xt", [IN, CPC * PC], mmdt, kind="ExternalInput").ap()
    out = nc.dram_tensor("out", [CPC * PC, OUT], f32, kind="ExternalOutput").ap()

    with tile.TileContext(nc) as tc:
        with (
            tc.tile_pool(name="xp", bufs=1) as xp,
            tc.tile_pool(name="wp", bufs=wbufs) as wp,
            tc.tile_pool(name="pp", bufs=4, space="PSUM") as pp,
            tc.tile_pool(name="op", bufs=3) as op,
        ):

            def body():
                G = cats_per_dma
                if interleave:
                    # p-outer row mapping: partition p holds IN rows
                    # KCH*p + s (s=0..KCH-1). Every DMA is contiguous per
                    # partition (8 KB weight runs, one single xT DMA); the
                    # contraction over s-subsets is a row permutation the
                    # matmul accumulation doesn't care about, as long as x
                    # and W use the same mapping.
                    xt4 = xp.tile([128, KCH, CPC * PC], mmdt, tag="x4")
                    nc.scalar.dma_start(
                        out=xt4[:], in_=xt.rearrange("(p s) c -> p s c", p=128)
                    )
                    lhs = lambda s, g: xt4[:, s, g * PC : (g + 1) * PC]
                else:
                    xts = []
                    for k in range(KCH):
                        t = xp.tile([128, CPC * PC], mmdt, tag=f"x{k}")
                        # ACT ring: keep SP HWDGE free for the weight stream
                        nc.scalar.dma_start(
                            out=t[:], in_=xt[k * 128 : (k + 1) * 128, :]
                        )
                        xts.append(t)
                    lhs = lambda s, g: xts[s][:, g * PC : (g + 1) * PC]
                for gp in range(0, CPC, G):
                    # Weight block [G cats] as SBUF [128, G, KCH, OUT]. G MB/DMA.
                    wtile = wp.tile([128, G, KCH, OUT], mmdt)
                    if interleave:
                        src = wt[gp * IN : (gp + G) * IN, :].rearrange(
                            "(g p s) o -> p g s o", p=128, s=KCH
                        )
                    else:
                        src = wt[gp * IN : (gp + G) * IN, :].rearrange(
                            "(g k p) o -> p g k o", p=128, k=KCH
                        )
                    nc.sync.dma_start(out=wtile[:], in_=src)
                    for gl in range(G):
                        g = gp + gl
                        ps = pp.tile([PC, OUT], f32)
                        for k in range(KCH):
                            nc.tensor.matmul(
                                ps[:],
                                lhsT=lhs(k, g),
                                rhs=wtile[:, gl, k, :],
                                start=(k == 0),
                                stop=(k == KCH - 1),
                            )
                        ot = op.tile([PC, OUT], f32)
                        nc.vector.tensor_copy(out=ot[:], in_=ps[:])
                        nc.scalar.dma_start(
                            out=out[g * PC : (g + 1) * PC, :], in_=ot[:]
                        )

            if loop_iters > 0:
                with tc.For_i(0, loop_iters, 1):
                    for _ in range(unroll):
                        body()
            else:
                body()
    nc.compile()
    return nc


def _prepare(x, selected_ids, weight):
    """Host-side shard prep. Returns (in_maps, meta) or (None, fallback_out)."""
    x = np.ascontiguousarray(np.asarray(x, dtype=np.float32))
    ids = np.asarray(selected_ids).astype(np.int64).ravel()
    weight = np.asarray(weight, dtype=np.float32)
    counts = np.bincount(ids, minlength=C)
    mx = int(counts.max())
    if mx > 128 or weight.shape != (C, OUT, IN) or x.shape != (B, IN):
        return None, None  # pathological skew / unexpected shape -> host path
    order = np.argsort(ids, kind="stable")
    x_sorted = x[order]
    offs = np.zeros(C + 1, np.int64)
    offs[1:] = np.cumsum(counts)
    PC = max(16, -(-mx // 16) * 16)  # round up to 16, at least 16
    wt_t = np.ascontiguousarray(weight.transpose(0, 2, 1))  # [C, IN, OUT]
    in_maps = []
    for core in range(NCORES):
        xt_k = np.zeros((IN, CPC * PC), np.float32)
        for gl in range(CPC):
            c = core * CPC + gl
            n = int(counts[c])
            if n:
                xt_k[:, gl * PC : gl * PC + n] = x_sorted[offs[c] : offs[c + 1]].T
        w_k = wt_t[core * CPC : (core + 1) * CPC].reshape(CPC * IN, OUT)
        in_maps.append({"wt": w_k, "xt": xt_k})
    meta = dict(PC=PC, counts=counts, offs=offs, order=order)
    return in_maps, meta


def _gather(results, meta):
    counts, offs, order, PC = (
        meta["counts"],
        meta["offs"],
        meta["order"],
        meta["PC"],
    )
    out_sorted = np.empty((B, OUT), np.float32)
    for core in range(NCORES):
        o = results[core]["out"]
        for gl in range(CPC):
            c = core * CPC + gl
            n = int(counts[c])
            if n:
                out_sorted[offs[c] : offs[c + 1]] = o[gl * PC : gl * PC + n]
    out_full = np.empty_like(out_sorted)
    out_full[order] = out_sorted
    return out_full


_LAST = {}  # debug/test introspection: last built nc + shard maps


def kernel(x, selected_ids, weight):
    in_maps, meta = _prepare(x, selected_ids, weight)
    if in_maps is None:
        ids = np.asarray(selected_ids).astype(np.int64).ravel()
        w = np.asarray(weight, dtype=np.float32)
        xx = np.asarray(x, dtype=np.float32)
        return np.einsum("boi,bi->bo", w[ids], xx).astype(np.float32)
    from concourse.bass_utils import run_bass_kernel_spmd

    nc = _build_nc(meta["PC"])
    _LAST.update(nc=nc, in_maps=in_maps, meta=meta)
    res = run_bass_kernel_spmd(nc, in_maps, core_ids=list(range(NCORES)))
    return _gather(res.results, meta)
